# revision 1
# baseline (speedup 1.0000x reference)
"""Trainium2 Bass kernel for LoRALayer: out = 2.0 * (x @ B) @ A.

x: [4, 4096, 4096] f32; A: [8, 4096] f32; B: [4096, 8] f32.
Sharding: data-parallel on the 16384 tokens across 8 cores (2048 each);
A/B replicated. Host-side prep (part of sharding): each core's x-shard is
shipped transposed (contraction dim on SBUF partitions) and split into
bf16 hi/lo halves (x = hi + lo exactly captures 16 mantissa bits); B and
2*A likewise. bf16 matmuls are ~4x cheaper than fp32 on the PE (single
pass + fast weight load), and the hi/lo compensation keeps ~1e-5 accuracy.

Per core, per TBLK-token block (f32 PSUM accumulation; 32-aligned row blocks
because engine partition bases must be multiples of 32):
  mm1 chain A: ps_a[40,TBLK] += [B_hi|0|B_lo]_c.T @ xh_c  (32 chunks; M-packed:
               rows 0-7 = x_hi@B_hi, rows 32-39 = x_hi@B_lo)
  mm1 chain B: ps_b[8,TBLK]  += B_hi_c.T @ xl_c           (x_lo@B_hi)
  y = ps_a[0:8] + ps_a[32:40] + ps_b  (DVE, f32); split y -> y_hi/y_lo bf16,
  pack K-wise as rows {0-7: y_hi, 32-39: y_hi, 64-71: y_lo} against
  A2pk rows {0-7: A_hi, 32-39: A_lo, 64-71: A_hi} (zeros elsewhere):
  mm2: out[128,512] = y_pack_sub[96,128].T @ A2pk[96,512]  (one MM = all 3
       correction terms; zero rows contribute nothing).
mm2 of block b-1 is interleaved with mm1 of block b (PE density for HAM);
input DMAs ride the sync HWDGE ring, output DMAs the scalar ring; PSUM->SBUF
copies alternate DVE/ACT.
"""

import numpy as np

P = 128
F_IN = 4096
F_OUT = 4096
RANK = 8
N_CORES = 8
SCALING = 2.0
TBLK = 256             # token block (mm1 rhs free dim, max 512 for f32 PSUM out)

_CACHE = {}


def _build_nc(T, F_in, F_out, R):
    """Build the single-core Bass program for a T-token shard."""
    from contextlib import ExitStack

    import concourse.mybir as mybir
    import concourse.tile as tile
    from concourse import bacc

    f32 = mybir.dt.float32
    bf16 = mybir.dt.bfloat16
    tblk = min(TBLK, T)
    CH = F_in // P          # feature chunks (32)
    NB = T // tblk          # token blocks (4)
    NSUB = tblk // P        # 128-token subtiles per block (4)
    NS = F_out // 512       # output column chunks (8)
    CGRP = min(8, CH)       # chunks per input sub-DMA (1MB bf16 granularity)
    NDMA = CH // CGRP       # input sub-DMAs per tensor per block (4)
    MM2_PER_BLK = NSUB * NS  # 32
    RB = 32                  # 32-aligned row blocks (engine partition bases)

    nc = bacc.Bacc("TRN2", target_bir_lowering=False, debug=False)

    xh_d = nc.dram_tensor(
        "xh", [NB, NDMA, P, CGRP * tblk], bf16, kind="ExternalInput"
    ).ap()
    xl_d = nc.dram_tensor(
        "xl", [NB, NDMA, P, CGRP * tblk], bf16, kind="ExternalInput"
    ).ap()
    bpk_d = nc.dram_tensor("Bpk", [P, CH * 2 * RB], bf16, kind="ExternalInput").ap()
    a2pk_d = nc.dram_tensor("A2pk", [3 * RB, F_out], bf16, kind="ExternalInput").ap()
    out_d = nc.dram_tensor("out", [T, F_out], f32, kind="ExternalOutput").ap()

    with tile.TileContext(nc) as tc, ExitStack() as ctx:
        cpool = ctx.enter_context(tc.tile_pool(name="const", bufs=1))
        xtpool = ctx.enter_context(tc.tile_pool(name="xt", bufs=2 * NDMA))
        ytpool = ctx.enter_context(tc.tile_pool(name="yt", bufs=3))
        opool = ctx.enter_context(tc.tile_pool(name="osb", bufs=3))
        y_pp = ctx.enter_context(tc.tile_pool(name="y_ps", bufs=2, space="PSUM"))
        o_pp = ctx.enter_context(tc.tile_pool(name="o_ps", bufs=4, space="PSUM"))

        bpk_sb = cpool.tile([P, CH * 2 * RB], bf16, tag="bpk_sb")
        nc.sync.dma_start(bpk_sb[:], bpk_d)
        apk_sb = cpool.tile([3 * RB, F_out], bf16, tag="apk_sb")
        nc.sync.dma_start(apk_sb[:], a2pk_d)

        blk_state = {}

        def emit_mm2(blk, idx):
            """idx in [0, MM2_PER_BLK): (sub, n) pair for block blk."""
            sub, n = divmod(idx, NS)
            y_pack, o_sbs = blk_state[blk]
            if n == 0:
                o_sbs[sub] = opool.tile(
                    [P, F_out], f32, tag="o_sb", name=f"o_sb_{blk}_{sub}"
                )
            o_sb = o_sbs[sub]
            o_ps = o_pp.tile([P, 512], f32, tag="o_ps")
            nc.tensor.matmul(
                o_ps[:],
                y_pack[:, sub * P:(sub + 1) * P],
                apk_sb[:, n * 512:(n + 1) * 512],
                start=True,
                stop=True,
            )
            if n % 2 == 0:
                nc.scalar.copy(o_sb[:, n * 512:(n + 1) * 512], o_ps[:])
            else:
                nc.vector.tensor_copy(o_sb[:, n * 512:(n + 1) * 512], o_ps[:])
            if n == NS - 1:
                trow = blk * tblk + sub * P
                nc.scalar.dma_start(out_d[trow:trow + P, :], o_sb[:])

        for blk in range(NB + 1):
            xhs, xls = [], []
            if blk < NB:
                for s in range(NDMA):
                    xh_sb = xtpool.tile([P, CGRP, tblk], bf16, tag="xh_sb")
                    nc.sync.dma_start(
                        xh_sb[:].rearrange("p c t -> p (c t)"), xh_d[blk, s]
                    )
                    xhs.append(xh_sb)
                    xl_sb = xtpool.tile([P, CGRP, tblk], bf16, tag="xl_sb")
                    nc.sync.dma_start(
                        xl_sb[:].rearrange("p c t -> p (c t)"), xl_d[blk, s]
                    )
                    xls.append(xl_sb)
                ps_a = y_pp.tile([RB + R, tblk], f32, tag="ps_a")
                ps_b = y_pp.tile([R, tblk], f32, tag="ps_b")

            # Interleave mm1 of this block 1:1 with mm2 of the previous block.
            n_steps = max(CH if blk < NB else 0, MM2_PER_BLK if blk > 0 else 0)
            for i in range(n_steps):
                if blk > 0 and i < MM2_PER_BLK:
                    emit_mm2(blk - 1, i)
                if blk < NB and i < CH:
                    c = i
                    nc.tensor.matmul(
                        ps_a[:],
                        bpk_sb[:, c * 2 * RB:c * 2 * RB + RB + R],
                        xhs[c // CGRP][:, c % CGRP, :],
                        start=(c == 0),
                        stop=(c == CH - 1),
                    )
                    nc.tensor.matmul(
                        ps_b[:],
                        bpk_sb[:, c * 2 * RB:c * 2 * RB + R],
                        xls[c // CGRP][:, c % CGRP, :],
                        start=(c == 0),
                        stop=(c == CH - 1),
                    )
            if blk > 0:
                del blk_state[blk - 1]
            if blk < NB:
                # y = hh + hl + lh (f32), then split into bf16 hi/lo and pack
                # K-wise as [y_hi; y_hi; y_lo] for the one-shot mm2.
                yt32 = ytpool.tile([R, tblk], f32, tag="yt32")
                nc.vector.tensor_copy(yt32[:], ps_a[:R, :])
                nc.vector.tensor_add(yt32[:], yt32[:], ps_a[RB:RB + R, :])
                nc.vector.tensor_add(yt32[:], yt32[:], ps_b[:])
                y_pack = ytpool.tile([3 * RB, tblk], bf16, tag="y_pack")
                nc.gpsimd.memset(y_pack[:], 0.0)
                nc.vector.tensor_copy(y_pack[:R, :], yt32[:])               # y_hi
                nc.vector.tensor_copy(y_pack[RB:RB + R, :], y_pack[:R, :])  # dup
                y_hi32 = ytpool.tile([R, tblk], f32, tag="y_hi32")
                nc.vector.tensor_copy(y_hi32[:], y_pack[:R, :])             # f32
                nc.vector.tensor_sub(y_pack[2 * RB:2 * RB + R, :], yt32[:], y_hi32[:])
                blk_state[blk] = (y_pack, {})

    nc.compile()
    return nc


def _pack_inputs(x2d, A, B, T_shard, F_in, R):
    """Shard x on tokens (transposed + bf16 hi/lo split); replicate B/A packs."""
    import ml_dtypes

    bf16 = ml_dtypes.bfloat16
    CH = F_in // P

    def split(m):
        hi = m.astype(bf16)
        lo = (m - hi.astype(np.float32)).astype(bf16)
        return hi, lo

    RB = 32
    R = B.shape[1]
    Bh, Bl = split(B.astype(np.float32))
    # chunk-major pack, 32-aligned: per chunk c of 2*RB cols:
    #   [0:R]=B_hi, [RB:RB+R]=B_lo, rest zero
    bpk = np.zeros((CH, P, 2 * RB), dtype=Bh.dtype)
    bpk[:, :, :R] = Bh.reshape(CH, P, R)
    bpk[:, :, RB:RB + R] = Bl.reshape(CH, P, R)
    bpk = np.ascontiguousarray(bpk.transpose(1, 0, 2).reshape(P, CH * 2 * RB))

    A2 = (SCALING * A).astype(np.float32)
    Ah, Al = split(A2)
    a2pk = np.zeros((3 * RB, A2.shape[1]), dtype=Ah.dtype)
    a2pk[:R] = Ah
    a2pk[RB:RB + R] = Al
    a2pk[2 * RB:2 * RB + R] = Ah
    a2pk = np.ascontiguousarray(a2pk)

    # device-DMA-friendly pack: [NB, NDMA, P, CGRP*tblk] so each sub-DMA
    # reads one contiguous per-partition run.
    T = T_shard
    tblk = min(TBLK, T)
    NB = T // tblk
    CGRP = min(8, CH)
    NDMA = CH // CGRP

    def pack(m):
        a = m.reshape(NDMA, CGRP, P, NB, tblk)
        a = a.transpose(3, 0, 2, 1, 4)
        return np.ascontiguousarray(a.reshape(NB, NDMA, P, CGRP * tblk))

    n_shards = x2d.shape[0] // T_shard
    in_maps = []
    for c in range(n_shards):
        xt = np.ascontiguousarray(x2d[c * T_shard:(c + 1) * T_shard].T)
        xh, xl = split(xt)
        in_maps.append(
            {"xh": pack(xh), "xl": pack(xl), "Bpk": bpk, "A2pk": a2pk}
        )
    return in_maps


def kernel(x, A, B):
    from concourse.bass_utils import run_bass_kernel_spmd

    x = np.asarray(x, dtype=np.float32)
    A = np.asarray(A, dtype=np.float32)
    B = np.asarray(B, dtype=np.float32)
    orig_shape = x.shape
    x2d = x.reshape(-1, F_IN)
    T_shard = x2d.shape[0] // N_CORES

    key = (T_shard, F_IN, F_OUT, RANK)
    if key not in _CACHE:
        _CACHE[key] = _build_nc(T_shard, F_IN, F_OUT, RANK)
    nc = _CACHE[key]

    in_maps = _pack_inputs(x2d, A, B, T_shard, F_IN, RANK)
    res = run_bass_kernel_spmd(nc, in_maps, core_ids=list(range(N_CORES)))
    out = np.concatenate([r["out"] for r in res.results], axis=0)
    return out.reshape(*orig_shape[:-1], F_OUT)



# revision 2
# speedup vs baseline: 1.5280x; 1.5280x over previous
"""Trainium2 Bass kernel for LoRALayer: out = 2.0 * (x @ B) @ A.

x: [4, 4096, 4096] f32; A: [8, 4096] f32; B: [4096, 8] f32.
Sharding: data-parallel on the 16384 tokens across 8 cores (2048 each);
A/B replicated. Host-side prep (part of sharding): each core's x-shard is
shipped transposed (contraction dim on SBUF partitions) as a single bf16
stream; B and 2*A likewise bf16. Output leaves the device as bf16 and the
host upconverts to f32 during the gather. This halves HBM traffic vs an
f32-out / hi+lo-in scheme (32 MiB/core total vs 64), and the kernel is
HBM-bound at ~358 GB/s/core. Accuracy: bf16 rounding of x dominates and
gives ~5e-3 absmax-rel error (gate 2e-2).

Per core (T=2048 tokens), per 512-token block:
  mm1: ps_y[8,512] += Bc.T @ xc over 32 feature chunks (f32 PSUM accum);
       y -> bf16 in SBUF.
  mm2 (interleaved with next block's mm1, one 128-token subtile per 8-chunk
       mm1 group): out[128,512] = y_sub[8,128].T @ A2[:,n*512:...] for the
       8 column chunks; PSUM->SBUF copies alternate ACT/DVE; out DMA bf16.
Input DMAs ride the sync HWDGE ring, output DMAs the scalar ring.
"""

import numpy as np

P = 128
F_IN = 4096
F_OUT = 4096
RANK = 8
N_CORES = 8
SCALING = 2.0
TBLK = 512             # token block (mm1 rhs free dim = one f32 PSUM bank)

_CACHE = {}


def _build_nc(T, F_in, F_out, R):
    """Build the single-core Bass program for a T-token shard."""
    from contextlib import ExitStack

    import concourse.mybir as mybir
    import concourse.tile as tile
    from concourse import bacc

    f32 = mybir.dt.float32
    bf16 = mybir.dt.bfloat16
    tblk = min(TBLK, T)
    CH = F_in // P          # feature chunks (32)
    NB = T // tblk          # token blocks (4)
    NSUB = tblk // P        # 128-token subtiles per block (4)
    NS = F_out // 512       # output column chunks (8)
    CGRP = CH // NSUB       # chunks per input sub-DMA (8 -> 1MB granularity)

    nc = bacc.Bacc("TRN2", target_bir_lowering=False, debug=False)

    xh_d = nc.dram_tensor(
        "xh", [NB, NSUB, P, CGRP * tblk], bf16, kind="ExternalInput"
    ).ap()
    bpk_d = nc.dram_tensor("Bpk", [P, CH * R], bf16, kind="ExternalInput").ap()
    a2_d = nc.dram_tensor("A2", [R, F_out], bf16, kind="ExternalInput").ap()
    out_d = nc.dram_tensor("out", [T, F_out], bf16, kind="ExternalOutput").ap()

    with tile.TileContext(nc) as tc, ExitStack() as ctx:
        cpool = ctx.enter_context(tc.tile_pool(name="const", bufs=1))
        xtpool = ctx.enter_context(tc.tile_pool(name="xt", bufs=2 * NSUB))
        ytpool = ctx.enter_context(tc.tile_pool(name="yt", bufs=2))
        opool = ctx.enter_context(tc.tile_pool(name="osb", bufs=3))
        y_pp = ctx.enter_context(tc.tile_pool(name="y_ps", bufs=2, space="PSUM"))
        o_pp = ctx.enter_context(tc.tile_pool(name="o_ps", bufs=4, space="PSUM"))

        bpk_sb = cpool.tile([P, CH * R], bf16, tag="bpk_sb")
        nc.sync.dma_start(bpk_sb[:], bpk_d)
        a2_sb = cpool.tile([R, F_out], bf16, tag="a2_sb")
        nc.sync.dma_start(a2_sb[:], a2_d)

        y_sbs = {}

        def emit_mm2(blk, sub):
            """One 128-token output subtile of block blk: 8 rank-8 matmuls."""
            y_sb = y_sbs[blk]
            o_sb = opool.tile([P, F_out], bf16, tag="o_sb")
            for n in range(NS):
                o_ps = o_pp.tile([P, 512], f32, tag="o_ps")
                nc.tensor.matmul(
                    o_ps[:],
                    y_sb[:, sub * P:(sub + 1) * P],
                    a2_sb[:, n * 512:(n + 1) * 512],
                    start=True,
                    stop=True,
                )
                if n % 2 == 0:
                    nc.scalar.copy(o_sb[:, n * 512:(n + 1) * 512], o_ps[:])
                else:
                    nc.vector.tensor_copy(o_sb[:, n * 512:(n + 1) * 512], o_ps[:])
            trow = blk * tblk + sub * P
            nc.scalar.dma_start(out_d[trow:trow + P, :], o_sb[:])

        for blk in range(NB + 1):
            xts = []
            if blk < NB:
                for s in range(NSUB):
                    x_sb = xtpool.tile([P, CGRP, tblk], bf16, tag="x_sb")
                    nc.sync.dma_start(
                        x_sb[:].rearrange("p c t -> p (c t)"), xh_d[blk, s]
                    )
                    xts.append(x_sb)
                ps_y = y_pp.tile([R, tblk], f32, tag="ps_y")

            # One mm2 subtile of the previous block per 8-chunk mm1 group:
            # PE alternates y-weight loads (128 cols) with cheap B loads.
            for g in range(NSUB):
                if blk > 0:
                    emit_mm2(blk - 1, g)
                if blk < NB:
                    for j in range(CGRP):
                        c = g * CGRP + j
                        nc.tensor.matmul(
                            ps_y[:],
                            bpk_sb[:, c * R:(c + 1) * R],
                            xts[g][:, j, :],
                            start=(c == 0),
                            stop=(c == CH - 1),
                        )
            if blk > 0:
                del y_sbs[blk - 1]
            if blk < NB:
                y_sb = ytpool.tile([R, tblk], bf16, tag="y_sb")
                nc.vector.tensor_copy(y_sb[:], ps_y[:])
                y_sbs[blk] = y_sb

    nc.compile()
    return nc


def _pack_inputs(x2d, A, B, T_shard, F_in, R):
    """Shard x on tokens (transposed, bf16); replicate B/A2 packs."""
    import ml_dtypes

    bf16 = ml_dtypes.bfloat16
    CH = F_in // P

    Bb = B.astype(np.float32).astype(bf16)
    bpk = np.ascontiguousarray(
        Bb.reshape(CH, P, R).transpose(1, 0, 2).reshape(P, CH * R)
    )
    a2 = np.ascontiguousarray((SCALING * A.astype(np.float32)).astype(bf16))

    T = T_shard
    tblk = min(TBLK, T)
    NB = T // tblk
    NSUB = tblk // P
    CGRP = CH // NSUB

    def pack(m):
        a = m.reshape(NSUB, CGRP, P, NB, tblk)
        a = a.transpose(3, 0, 2, 1, 4)
        return np.ascontiguousarray(a.reshape(NB, NSUB, P, CGRP * tblk))

    n_shards = x2d.shape[0] // T_shard
    in_maps = []
    for c in range(n_shards):
        xt = np.ascontiguousarray(x2d[c * T_shard:(c + 1) * T_shard].T)
        in_maps.append({"xh": pack(xt.astype(bf16)), "Bpk": bpk, "A2": a2})
    return in_maps


def kernel(x, A, B):
    from concourse.bass_utils import run_bass_kernel_spmd

    x = np.asarray(x, dtype=np.float32)
    A = np.asarray(A, dtype=np.float32)
    B = np.asarray(B, dtype=np.float32)
    orig_shape = x.shape
    x2d = x.reshape(-1, F_IN)
    T_shard = x2d.shape[0] // N_CORES

    key = (T_shard, F_IN, F_OUT, RANK)
    if key not in _CACHE:
        _CACHE[key] = _build_nc(T_shard, F_IN, F_OUT, RANK)
    nc = _CACHE[key]

    in_maps = _pack_inputs(x2d, A, B, T_shard, F_IN, RANK)
    res = run_bass_kernel_spmd(nc, in_maps, core_ids=list(range(N_CORES)))
    out = np.concatenate(
        [np.asarray(r["out"], dtype=np.float32) for r in res.results], axis=0
    )
    return out.reshape(*orig_shape[:-1], F_OUT)


# revision 6
# speedup vs baseline: 1.6949x; 1.1093x over previous
"""Trainium2 Bass kernel for LoRALayer: out = 2.0 * (x @ B) @ A.

x: [4, 4096, 4096] f32; A: [8, 4096] f32; B: [4096, 8] f32.
Sharding: data-parallel on the 16384 tokens across 8 cores (2048 each);
A/B replicated. Host-side prep (part of sharding): each core's x-shard is
shipped transposed (contraction dim on SBUF partitions) as a single bf16
stream; B and 2*A likewise bf16. Output leaves the device as bf16 and the
host upconverts to f32 during the gather. Total HBM traffic is 32 MiB/core
(the kernel is HBM-bound); bf16 rounding of x dominates the ~7e-3
absmax-rel error (gate 2e-2).

Per core (T=2048), per 256-token block, PE work is packed with tile_position
concurrency so the PE never gates the DMA streams:
  mm1 (2x col-tiled): even feature chunks accumulate into ps_y[0:8],
      odd chunks into ps_y[32:40] (independent PE column groups, separate
      xbus streams) -> 16 rounds of 2 concurrent 128x8x256 matmuls.
  y   = even+odd strips, split into two 128-token halves at partition
      bases 0/32 (bf16), matching A2 replicated at bases 0/32.
  mm2 (2x row-tiled): per 512-col chunk of A2, two concurrent rank-8
      matmuls (row groups 0/1) -> two PSUM banks; ACT copies subtile 0,
      DVE copies subtile 1 (PSUM->SBUF bf16).
Input DMAs ride the sync HWDGE ring, output DMAs the gpsimd (SWDGE) ring
so trigger issue never serializes with copies.
"""

import numpy as np

P = 128
F_IN = 4096
F_OUT = 4096
RANK = 8
N_CORES = 8
SCALING = 2.0
TBLK = 256             # token block: 2 subtiles of 128 tokens

_CACHE = {}


def _build_nc(T, F_in, F_out, R):
    """Build the single-core Bass program for a T-token shard."""
    from contextlib import ExitStack

    import concourse.mybir as mybir
    import concourse.tile as tile
    from concourse import bacc

    f32 = mybir.dt.float32
    bf16 = mybir.dt.bfloat16
    tblk = min(TBLK, T)
    CH = F_in // P          # feature chunks (32)
    NB = T // tblk          # token blocks (8)
    NSUB = tblk // P        # 128-token subtiles per block (2)
    NS = F_out // 512       # output column chunks (8)
    CGRP = CH // NSUB       # chunks per input sub-DMA (16 -> 1MB granularity)
    RB = 32                 # partition-base alignment for engine APs

    nc = bacc.Bacc("TRN2", target_bir_lowering=False, debug=False)

    xh_d = nc.dram_tensor(
        "xh", [NB, NSUB, P, CGRP * tblk], bf16, kind="ExternalInput"
    ).ap()
    bpk_d = nc.dram_tensor("Bpk", [P, CH * R], bf16, kind="ExternalInput").ap()
    a2_d = nc.dram_tensor("A2", [R, F_out], bf16, kind="ExternalInput").ap()
    out_d = nc.dram_tensor("out", [T, F_out], bf16, kind="ExternalOutput").ap()

    with tile.TileContext(nc) as tc, ExitStack() as ctx:
        cpool = ctx.enter_context(tc.tile_pool(name="const", bufs=1))
        xtpool = ctx.enter_context(tc.tile_pool(name="xt", bufs=2 * NSUB))
        ypool = ctx.enter_context(tc.tile_pool(name="yt", bufs=2))
        opool = ctx.enter_context(tc.tile_pool(name="osb", bufs=4))
        y_pp = ctx.enter_context(tc.tile_pool(name="y_ps", bufs=2, space="PSUM"))
        o_pp = ctx.enter_context(tc.tile_pool(name="o_ps", bufs=2, space="PSUM"))

        bpk_sb = cpool.tile([P, CH * R], bf16, tag="bpk_sb")
        nc.sync.dma_start(bpk_sb[:], bpk_d)
        # A2 replicated at partition bases 0 and 32 for the row-tiled mm2.
        a2_sb = cpool.tile([RB + R, F_out], bf16, tag="a2_sb")
        nc.sync.dma_start(a2_sb[:R, :], a2_d)
        nc.sync.dma_start(a2_sb[RB:RB + R, :], a2_d)

        state = {}

        for blk in range(NB + 1):
            xts = []
            if blk < NB:
                for s in range(NSUB):
                    x_sb = xtpool.tile([P, CGRP, tblk], bf16, tag="x_sb")
                    nc.sync.dma_start(
                        x_sb[:].rearrange("p c t -> p (c t)"), xh_d[blk, s]
                    )
                    xts.append(x_sb)
                ps_y = y_pp.tile([RB + R, 512], f32, tag="ps_y")

            for h in range(NSUB):
                if blk > 0:
                    y_pk, o_sbs = state[blk - 1]
                    if h == 0:
                        o_sbs.append(opool.tile(
                            [P, F_out], bf16, tag="o_sb0",
                            name=f"o_sb0_{blk}",
                        ))
                        o_sbs.append(opool.tile(
                            [P, F_out], bf16, tag="o_sb1",
                            name=f"o_sb1_{blk}",
                        ))
                    for n in range(h * NS // NSUB, (h + 1) * NS // NSUB):
                        cs = slice(n * 512, (n + 1) * 512)
                        o_ps0 = o_pp.tile([P, 512], f32, tag="o_ps0")
                        o_ps1 = o_pp.tile([P, 512], f32, tag="o_ps1")
                        nc.tensor.matmul(
                            o_ps0[:], y_pk[:R, :], a2_sb[:R, cs],
                            start=True, stop=True,
                        )
                        nc.tensor.matmul(
                            o_ps1[:], y_pk[RB:RB + R, :], a2_sb[RB:RB + R, cs],
                            start=True, stop=True,
                        )
                        nc.scalar.copy(o_sbs[0][:, cs], o_ps0[:])
                        nc.vector.tensor_copy(o_sbs[1][:, cs], o_ps1[:])
                if blk < NB:
                    for r in range(h * CGRP // 2, (h + 1) * CGRP // 2):
                        c0, c1 = 2 * r, 2 * r + 1
                        nc.tensor.matmul(
                            ps_y[:R, :tblk],
                            bpk_sb[:, c0 * R:(c0 + 1) * R],
                            xts[c0 // CGRP][:, c0 % CGRP, :],
                            start=(r == 0), stop=(r == CH // 2 - 1),
                        )
                        nc.tensor.matmul(
                            ps_y[RB:RB + R, :tblk],
                            bpk_sb[:, c1 * R:(c1 + 1) * R],
                            xts[c1 // CGRP][:, c1 % CGRP, :],
                            start=(r == 0), stop=(r == CH // 2 - 1),
                        )
            if blk > 0:
                _, o_sbs = state.pop(blk - 1)
                trow = (blk - 1) * tblk
                nc.gpsimd.dma_start(out_d[trow:trow + P, :], o_sbs[0][:])
                nc.gpsimd.dma_start(out_d[trow + P:trow + 2 * P, :], o_sbs[1][:])
            if blk < NB:
                # y = even+odd col-tile strips; two 128-token halves at
                # partition bases 0/32 (the mm2 row-tile weight layout).
                # DVE reads at most one PSUM operand -> stage strip 0 in SBUF.
                yt = ypool.tile([R, tblk], f32, tag="yt")
                nc.vector.tensor_copy(yt[:], ps_y[:R, :tblk])
                y_pk = ypool.tile([RB + R, P], bf16, tag="y_pk")
                nc.vector.tensor_add(
                    y_pk[:R, :], yt[:, :P], ps_y[RB:RB + R, :P]
                )
                nc.vector.tensor_add(
                    y_pk[RB:RB + R, :], yt[:, P:tblk], ps_y[RB:RB + R, P:tblk]
                )
                state[blk] = (y_pk, [])

    nc.compile()
    return nc


def _pack_inputs(x2d, A, B, T_shard, F_in, R):
    """Shard x on tokens (transposed, bf16); replicate B/A2 packs."""
    import ml_dtypes

    bf16 = ml_dtypes.bfloat16
    CH = F_in // P

    Bb = B.astype(np.float32).astype(bf16)
    bpk = np.ascontiguousarray(
        Bb.reshape(CH, P, R).transpose(1, 0, 2).reshape(P, CH * R)
    )
    a2 = np.ascontiguousarray((SCALING * A.astype(np.float32)).astype(bf16))

    T = T_shard
    tblk = min(TBLK, T)
    NB = T // tblk
    NSUB = tblk // P
    CGRP = CH // NSUB

    def pack(m):
        a = m.reshape(NSUB, CGRP, P, NB, tblk)
        a = a.transpose(3, 0, 2, 1, 4)
        return np.ascontiguousarray(a.reshape(NB, NSUB, P, CGRP * tblk))

    n_shards = x2d.shape[0] // T_shard
    in_maps = []
    for c in range(n_shards):
        xt = np.ascontiguousarray(x2d[c * T_shard:(c + 1) * T_shard].T)
        in_maps.append({"xh": pack(xt.astype(bf16)), "Bpk": bpk, "A2": a2})
    return in_maps


def kernel(x, A, B):
    from concourse.bass_utils import run_bass_kernel_spmd

    x = np.asarray(x, dtype=np.float32)
    A = np.asarray(A, dtype=np.float32)
    B = np.asarray(B, dtype=np.float32)
    orig_shape = x.shape
    x2d = x.reshape(-1, F_IN)
    T_shard = x2d.shape[0] // N_CORES

    key = (T_shard, F_IN, F_OUT, RANK)
    if key not in _CACHE:
        _CACHE[key] = _build_nc(T_shard, F_IN, F_OUT, RANK)
    nc = _CACHE[key]

    in_maps = _pack_inputs(x2d, A, B, T_shard, F_IN, RANK)
    res = run_bass_kernel_spmd(nc, in_maps, core_ids=list(range(N_CORES)))
    out = np.concatenate(
        [np.asarray(r["out"], dtype=np.float32) for r in res.results], axis=0
    )
    return out.reshape(*orig_shape[:-1], F_OUT)
